# revision 14
# baseline (speedup 1.0000x reference)
"""Multi-head attention (B=2, S=2048, D=1024, H=16) on 8 TRN2 NeuronCores.

Sharding: batch x head-group. Core c handles batch b = c//4 and heads
h0 = (c%4)*4 .. h0+4 (Megatron-style: wq/wk/wv column-split, wo row-split).
Each core computes, for its 4 heads:
  Q.T, K.T = [256, 2048] projections (dims on partitions, tokens free)
  V       = [2048, 256]  (tokens on partitions)
  per head: scores (QK layout) -> exp (+row sums via accum_out) -> attn out
            scores.T (T layout) -> exp -> ctx.T accumulation
            ctx.T scaled by 1/Z (broadcast via K=1 PE matmul)
  out_partial = Ctx.T.T @ Wo_rows  (host sums the 4 partials per batch)

Softmax is computed without the max-subtraction (scores are O(1) for this
problem; exp cannot overflow) which matches jax.nn.softmax exactly up to fp
rounding. wk_b is mathematically a no-op for both outputs (constant per-row
shift of scores); wv_b and wo_b are affine post-terms folded in on the host;
wq_b is applied on-device to Q.T.

Heavy matmuls run in MM_DTYPE (default bf16, fp32 PSUM accumulation).
"""

import os
import sys
import types
from contextlib import ExitStack

import numpy as np

import concourse.bacc as bacc
import concourse.bass as bass
import concourse.mybir as mybir
import concourse.tile as tile
from concourse import bass_utils

S = 2048
DM = 1024
HD = 64
NHL = 4          # heads per core
LD = NHL * HD    # 256 local projection dims
NCORES = 8
KT = 8           # d_model tiles of 128
QT = 16          # token tiles of 128
NCH = 4          # token chunks of 512

F32 = mybir.dt.float32
F32R = mybir.dt.float32r
EXP = mybir.ActivationFunctionType.Exp
AXX = mybir.AxisListType.X
ts = bass.ts

MM_DTYPE = os.environ.get("MHA_MM_DTYPE", "bf16")
MF = {"bf16": mybir.dt.bfloat16, "f32r": F32R, "f32": F32}[MM_DTYPE]
NP_MF = mybir.dt.np(MF)


def _r(ap):
    return ap


def _install_ntff_hook():
    """Allow BASS_TRACE=1 profiling under axon (optional)."""
    try:
        from antenv.axon_hooks import get_axon_ntff_profile_hook  # noqa: F401
        return
    except ImportError:
        pass
    try:
        from trn_agent_boot.trn_boot import _ntff_profile_via_ctypes

        hook = _ntff_profile_via_ctypes("/opt/axon/libaxon_pjrt.so")
        mod = types.ModuleType("antenv.axon_hooks")
        mod.get_axon_ntff_profile_hook = lambda: hook
        sys.modules["antenv.axon_hooks"] = mod
    except Exception:
        pass


def _emit(nc, tc, d):
    with ExitStack() as ctx:
        cp = ctx.enter_context(tc.tile_pool(name="const", bufs=1))
        pp = ctx.enter_context(tc.tile_pool(name="persist", bufs=1))

        # ---- constants ----
        wqt = [cp.tile([128, LD], MF, name=f"wqt{k}", tag=f"wqt{k}") for k in range(KT)]
        wkt = [cp.tile([128, LD], MF, name=f"wkt{k}", tag=f"wkt{k}") for k in range(KT)]
        wvt = [cp.tile([128, LD], MF, name=f"wvt{k}", tag=f"wvt{k}") for k in range(KT)]
        wo = [cp.tile([128, DM], MF, name=f"wo{k}", tag=f"wo{k}") for k in range(2)]
        bqt = [cp.tile([128, 1], F32, name=f"bqt{m}", tag=f"bqt{m}") for m in range(2)]
        ident = cp.tile([128, 128], F32, name="ident", tag="ident")
        ones1 = cp.tile([1, HD], F32, name="ones1", tag="ones1")
        hsrc = cp.tile([128, 512], MF, name="hsrc", tag="hsrc")
        warm = cp.tile([128, 1], F32, name="warm", tag="warm")
        for k in range(KT):
            nc.sync.dma_start(wkt[k][:], d["wk_t"][ts(k, 128), :])
        for k in range(KT):
            nc.sync.dma_start(wqt[k][:], d["wq_t"][ts(k, 128), :])
        for k in range(2):
            nc.sync.dma_start(bqt[k][:], d["bq"][ts(k, 128), :])
        for k in range(KT):
            nc.sync.dma_start(wvt[k][:], d["wv_t"][ts(k, 128), :])
        for k in range(2):
            nc.sync.dma_start(wo[k][:], d["wo_r"][ts(k, 128), :])
        nc.sync.dma_start(ident[:], d["ident"][:, :])
        nc.sync.dma_start(ones1[:], d["ones1"][:, :])
        nc.vector.memset(hsrc[:], 0.001)
        # preload the Exp table set during the projection phase
        nc.vector.memset(warm[:], 0.0)
        nc.scalar.activation(warm[:], warm[:], EXP)

        # ---- persistent activations ----
        QTm = [pp.tile([128, S], MF, name=f"QT{m}", tag=f"QT{m}") for m in range(2)]
        KTm = [pp.tile([128, S], MF, name=f"KT{m}", tag=f"KT{m}") for m in range(2)]
        V = [pp.tile([128, LD], MF, name=f"V{j}", tag=f"V{j}") for j in range(QT)]
        CT = [pp.tile([128, S], MF, name=f"CT{m}", tag=f"CT{m}") for m in range(2)]
        IZ = [pp.tile([128, QT], F32, name=f"IZ{h}", tag=f"IZ{h}") for h in range(NHL)]

        xt = ctx.enter_context(tc.tile_pool(name="xt", bufs=10))
        izp = ctx.enter_context(tc.tile_pool(name="izp", bufs=2))
        stage = ctx.enter_context(tc.tile_pool(name="stage", bufs=6))
        expp = ctx.enter_context(tc.tile_pool(name="expp", bufs=3))
        smal = ctx.enter_context(tc.tile_pool(name="smal", bufs=2))
        zp = ctx.enter_context(tc.tile_pool(name="zp", bufs=3))

        def heater(pool, tag, nmm, label, bufs=None):
            # dense dummy matmul burst: forces the PE HAM monitor back to
            # the unthrottled clock after a pipeline bubble
            hp = pool.tile([128, 512], F32, name=f"heat{label}", tag=tag, bufs=bufs)
            for i in range(nmm):
                nc.tensor.matmul(
                    hp[:, 0:512], hsrc[:, 0:128], hsrc[:],
                    start=(i == 0), stop=(i == nmm - 1), skip_group_check=True,
                )

        # ================= Phase B: Q.T / K.T projections =================
        with tc.tile_pool(name="psb", bufs=2, space="PSUM") as psb:
            for name, dx, wt, out_t, bias in (
                ("k", "xk_t", wkt, KTm, None),
                ("q", "xq_t", wqt, QTm, bqt),
            ):
                xts = []
                for k in range(KT):
                    t = xt.tile([128, S], MF, name=f"x{name}{k}", tag="xt")
                    nc.sync.dma_start(t[:], d[dx][ts(k, 128), :])
                    xts.append(t)
                for n in range(NCH):
                    for m in range(2):
                        ps = psb.tile([128, 512], F32, name=f"ps{name}{n}{m}", tag="psq")
                        for k in range(KT):
                            nc.tensor.matmul(
                                ps[:],
                                wt[k][:, ts(m, 128)],
                                xts[k][:, ts(n, 512)],
                                start=(k == 0),
                                stop=(k == KT - 1),
                            )
                        if bias is not None:
                            nc.vector.tensor_scalar_add(
                                out_t[m][:, ts(n, 512)], ps[:], bias[m][:]
                            )
                        else:
                            nc.vector.tensor_copy(out_t[m][:, ts(n, 512)], ps[:])

        # ====== Phase C: alternating C1 head-pair / C3 head sections ======
        def v_units(pool):
            # B3: V = [2048, 256]; borrows pss-tag PSUM slots
            xvs = []
            for k in range(KT):
                t = xt.tile([128, S], MF, name=f"xv{k}", tag="xt")
                nc.sync.dma_start(t[:], d["xv_t"][ts(k, 128), :])
                xvs.append(t)
            for j in range(QT):
                ps = pool.tile([128, 1024], F32, name=f"psv{j}", tag="pss")
                for k in range(KT):
                    nc.tensor.matmul(
                        ps[:, 0:LD],
                        xvs[k][:, ts(j, 128)],
                        wvt[k][:],
                        start=(k == 0),
                        stop=(k == KT - 1),
                    )
                nc.vector.tensor_copy(V[j][:], ps[:, 0:LD])
                yield

        def c1_units(hp, pool):
            # heads ha (array rows 0-63) and hb (rows 64-127), packed
            ha, hb = 2 * hp, 2 * hp + 1
            qta = QTm[hp][0:64, :]
            kta = KTm[hp][0:64, :]
            qtb = QTm[hp][64:128, :]
            ktb = KTm[hp][64:128, :]
            for q in range(QT):
                sta = stage.tile([128, S], F32, name=f"st{ha}_{q}", tag="stage")
                stb = stage.tile([128, S], F32, name=f"st{hb}_{q}", tag="stage")
                zpa = zp.tile([128, 2], F32, name=f"zpa{ha}_{q}", tag="zpa")
                zpb = zp.tile([128, 2], F32, name=f"zpb{hb}_{q}", tag="zpb")
                for half in range(2):
                    psa = pool.tile([128, 1024], F32, name=f"psa{ha}{q}{half}", tag="pss")
                    psb = pool.tile([128, 1024], F32, name=f"psb{hb}{q}{half}", tag="pss")
                    for kc in range(2):
                        kk = half * 2 + kc
                        nc.tensor.matmul(
                            psa[:, ts(kc, 512)],
                            qta[:, ts(q, 128)],
                            kta[:, ts(kk, 512)],
                            start=True,
                            stop=True,
                        )
                        nc.tensor.matmul(
                            psb[:, ts(kc, 512)],
                            qtb[:, ts(q, 128)],
                            ktb[:, ts(kk, 512)],
                            start=True,
                            stop=True,
                        )
                    nc.scalar.activation(
                        sta[:, ts(half, 1024)], psa[:], EXP,
                        scale=0.125, accum_out=zpa[:, half : half + 1],
                    )
                    nc.scalar.activation(
                        stb[:, ts(half, 1024)], psb[:], EXP,
                        scale=0.125, accum_out=zpb[:, half : half + 1],
                    )
                for h, stx, zpx in ((ha, sta, zpa), (hb, stb, zpb)):
                    z1 = zp.tile([128, 1], F32, name=f"z1{h}_{q}", tag="z1")
                    nc.vector.reduce_sum(z1[:], zpx[:], axis=AXX)
                    nc.vector.reciprocal(IZ[h][:, q : q + 1], z1[:])
                    nc.vector.tensor_scalar_mul(stx[:], stx[:], IZ[h][:, q : q + 1])
                    nc.sync.dma_start(
                        d["attn_p"][h * S + q * 128 : h * S + (q + 1) * 128, :],
                        stx[:],
                    )
                yield

        def c3_head(h, pool):
            po = 64 * (h % 2)
            qth = QTm[h // 2][po : po + 64, :]
            kth = KTm[h // 2][po : po + 64, :]

            heater(pool, "psst", 16, f"c3{h}", bufs=3)

            # 1/Z -> [1, S] line (transpose + sbuf->sbuf reshape)
            pst = pool.tile([16, 128], F32, name=f"pst{h}", tag="pssm")
            nc.tensor.transpose(pst[:], IZ[h][:], ident[:])
            izrow = smal.tile([16, 128], F32, name=f"izrow{h}", tag="izrow")
            nc.vector.tensor_copy(izrow[:], pst[:])
            izline = izp.tile([1, S], F32, name=f"izline{h}", tag="izline")
            for rr in range(QT):
                nc.sync.dma_start(izline[0:1, ts(rr, 128)], izrow[rr : rr + 1, :])

            for n in range(NCH):
                pctx = pool.tile([64, 512], F32, name=f"pctx{h}{n}", tag="psctx")
                for jj in range(QT // 2):
                    pstt = pool.tile([128, 1024], F32, name=f"pstt{h}{n}{jj}", tag="psst", bufs=3)
                    for u in range(2):
                        j = 2 * jj + u
                        nc.tensor.matmul(
                            pstt[:, ts(u, 512)],
                            kth[:, ts(j, 128)],
                            qth[:, ts(n, 512)],
                            start=True,
                            stop=True,
                        )
                    et = expp.tile([128, 1024], MF, name=f"et{h}{n}{jj}", tag="expt")
                    nc.scalar.activation(et[:], pstt[:], EXP, scale=0.125)
                    for u in range(2):
                        j = 2 * jj + u
                        nc.tensor.matmul(
                            pctx[:],
                            V[j][:, ts(h, HD)],
                            et[:, ts(u, 512)],
                            start=(j == 0),
                            stop=(j == QT - 1),
                        )
                pbc = pool.tile([64, 512], F32, name=f"pbc{h}{n}", tag="pssm")
                nc.tensor.matmul(
                    pbc[:],
                    ones1[:],
                    izline[0:1, ts(n, 512)],
                    start=True,
                    stop=True,
                )
                bcs = smal.tile([64, 512], F32, name=f"bcs{h}{n}", tag="bcast")
                nc.vector.tensor_copy(bcs[:], pbc[:])
                nc.vector.tensor_mul(
                    CT[h // 2][po : po + 64, ts(n, 512)], pctx[:], bcs[:]
                )

        with tc.tile_pool(name="psc1a", bufs=4, space="PSUM") as pool:
            heater(pool, "pss", 24, "c1a")
            gv = v_units(pool)
            g1 = c1_units(0, pool)
            done_v = done_1 = False
            while not (done_v and done_1):
                if next(gv, StopIteration) is StopIteration:
                    done_v = True
                if next(g1, StopIteration) is StopIteration:
                    done_1 = True
            heater(pool, "pss", 16, "c1b")
            for _ in c1_units(1, pool):
                pass
        with tc.tile_pool(name="psc3a", bufs=1, space="PSUM") as pool:
            for h in range(NHL):
                c3_head(h, pool)

        # ================= Phase D: output projection =================
        with tc.tile_pool(name="psd", bufs=2, space="PSUM") as psd:
            for mt in range(QT):
                ost = smal.tile([128, DM], F32, name=f"ost{mt}", tag="outst")
                for nn in range(2):
                    pso = psd.tile([128, 512], F32, name=f"pso{mt}{nn}", tag="pso")
                    for kk in range(2):
                        nc.tensor.matmul(
                            pso[:],
                            CT[kk][:, ts(mt, 128)],
                            wo[kk][:, ts(nn, 512)],
                            start=(kk == 0),
                            stop=(kk == 1),
                        )
                    nc.vector.tensor_copy(ost[:, ts(nn, 512)], pso[:])
                nc.sync.dma_start(d["out_p"][ts(mt, 128), :], ost[:])


_NC = None


def _build():
    global _NC
    if _NC is not None:
        return _NC
    nc = bacc.Bacc("TRN2", target_bir_lowering=False, debug=False)
    d = {}
    for name, shape, dt_ in (
        ("xq_t", [DM, S], MF),
        ("xk_t", [DM, S], MF),
        ("xv_t", [DM, S], MF),
        ("wq_t", [DM, LD], MF),
        ("wk_t", [DM, LD], MF),
        ("wv_t", [DM, LD], MF),
        ("wo_r", [LD, DM], MF),
        ("bq", [LD, 1], F32),
        ("ident", [128, 128], F32),
        ("ones1", [1, HD], F32),
    ):
        d[name] = nc.dram_tensor(name, shape, dt_, kind="ExternalInput").ap()
    d["attn_p"] = nc.dram_tensor("attn_p", [NHL * S, S], F32, kind="ExternalOutput").ap()
    d["out_p"] = nc.dram_tensor("out_p", [S, DM], F32, kind="ExternalOutput").ap()

    with tile.TileContext(nc) as tc:
        _emit(nc, tc, d)
    nc.compile()
    _NC = nc
    return nc


def make_in_maps(query, key, value, wq_w, wq_b, wk_w, wk_b, wv_w, wv_b, wo_w, wo_b):
    f = np.float32
    ident = np.eye(128, dtype=f)
    ones1 = np.ones((1, HD), dtype=f)
    xt = {}
    for b in range(2):
        xt[("q", b)] = np.ascontiguousarray(np.asarray(query[b], dtype=f).T).astype(NP_MF)
        xt[("k", b)] = np.ascontiguousarray(np.asarray(key[b], dtype=f).T).astype(NP_MF)
        xt[("v", b)] = np.ascontiguousarray(np.asarray(value[b], dtype=f).T).astype(NP_MF)
    wqT = np.asarray(wq_w, dtype=f).T
    wkT = np.asarray(wk_w, dtype=f).T
    wvT = np.asarray(wv_w, dtype=f).T
    in_maps = []
    for c in range(NCORES):
        b, hg = divmod(c, 4)
        sl = slice(hg * LD, (hg + 1) * LD)
        in_maps.append(
            {
                "xq_t": xt[("q", b)],
                "xk_t": xt[("k", b)],
                "xv_t": xt[("v", b)],
                "wq_t": np.ascontiguousarray(wqT[:, sl]).astype(NP_MF),
                "wk_t": np.ascontiguousarray(wkT[:, sl]).astype(NP_MF),
                "wv_t": np.ascontiguousarray(wvT[:, sl]).astype(NP_MF),
                "wo_r": np.ascontiguousarray(np.asarray(wo_w, dtype=f)[:, sl].T).astype(NP_MF),
                "bq": np.ascontiguousarray(np.asarray(wq_b, dtype=f)[sl]).reshape(LD, 1),
                "ident": ident,
                "ones1": ones1,
            }
        )
    return in_maps


def kernel(query, key, value, wq_w, wq_b, wk_w, wk_b, wv_w, wv_b, wo_w, wo_b):
    _install_ntff_hook()
    nc = _build()
    in_maps = make_in_maps(
        query, key, value, wq_w, wq_b, wk_w, wk_b, wv_w, wv_b, wo_w, wo_b
    )
    res = bass_utils.run_bass_kernel_spmd(nc, in_maps, core_ids=list(range(NCORES)))
    kernel.last_exec_time_ns = res.exec_time_ns

    f = np.float32
    B, H = 2, 16
    attn = np.empty((B, H, S, S), dtype=f)
    out = np.zeros((B, S, DM), dtype=f)
    for c in range(NCORES):
        b, hg = divmod(c, 4)
        attn[b, hg * NHL : (hg + 1) * NHL] = res.results[c]["attn_p"].reshape(NHL, S, S)
        out[b] += res.results[c]["out_p"]
    # host-folded affine terms: value bias through wo, and wo bias
    out += (np.asarray(wv_b, dtype=f) @ np.asarray(wo_w, dtype=f).T + np.asarray(wo_b, dtype=f))[None, None, :]
    return out, attn


# revision 15
# speedup vs baseline: 1.1606x; 1.1606x over previous
"""Multi-head attention (B=2, S=2048, D=1024, H=16) on 8 TRN2 NeuronCores.

Sharding: batch x head-group. Core c handles batch b = c//4 and heads
h0 = (c%4)*4 .. h0+4 (Megatron-style: wq/wk/wv column-split, wo row-split).
Each core computes, for its 4 heads:
  Q.T, K.T = [256, 2048] projections (dims on partitions, tokens free)
  V       = [2048, 256]  (tokens on partitions)
  per head: scores (QK layout) -> exp (+row sums via accum_out) -> attn out
            scores.T (T layout) -> exp -> ctx.T accumulation
            ctx.T scaled by 1/Z (broadcast via K=1 PE matmul)
  out_partial = Ctx.T.T @ Wo_rows  (host sums the 4 partials per batch)

Softmax is computed without the max-subtraction (scores are O(1) for this
problem; exp cannot overflow) which matches jax.nn.softmax exactly up to fp
rounding. wk_b is mathematically a no-op for both outputs (constant per-row
shift of scores); wv_b and wo_b are affine post-terms folded in on the host;
wq_b is applied on-device to Q.T.

Heavy matmuls run in MM_DTYPE (default bf16, fp32 PSUM accumulation).
"""

import os
import sys
import types
from contextlib import ExitStack

import numpy as np

import concourse.bacc as bacc
import concourse.bass as bass
import concourse.mybir as mybir
import concourse.tile as tile
from concourse import bass_utils

S = 2048
DM = 1024
HD = 64
NHL = 4          # heads per core
LD = NHL * HD    # 256 local projection dims
NCORES = 8
KT = 8           # d_model tiles of 128
QT = 16          # token tiles of 128
NCH = 4          # token chunks of 512

F32 = mybir.dt.float32
F32R = mybir.dt.float32r
EXP = mybir.ActivationFunctionType.Exp
AXX = mybir.AxisListType.X
ts = bass.ts

MM_DTYPE = os.environ.get("MHA_MM_DTYPE", "bf16")
MF = {"bf16": mybir.dt.bfloat16, "f32r": F32R, "f32": F32}[MM_DTYPE]
NP_MF = mybir.dt.np(MF)


def _r(ap):
    return ap


def _install_ntff_hook():
    """Allow BASS_TRACE=1 profiling under axon (optional)."""
    try:
        from antenv.axon_hooks import get_axon_ntff_profile_hook  # noqa: F401
        return
    except ImportError:
        pass
    try:
        from trn_agent_boot.trn_boot import _ntff_profile_via_ctypes

        hook = _ntff_profile_via_ctypes("/opt/axon/libaxon_pjrt.so")
        mod = types.ModuleType("antenv.axon_hooks")
        mod.get_axon_ntff_profile_hook = lambda: hook
        sys.modules["antenv.axon_hooks"] = mod
    except Exception:
        pass


def _emit(nc, tc, d):
    with ExitStack() as ctx:
        cp = ctx.enter_context(tc.tile_pool(name="const", bufs=1))
        pp = ctx.enter_context(tc.tile_pool(name="persist", bufs=1))

        # ---- constants ----
        wqt = [cp.tile([128, LD], MF, name=f"wqt{k}", tag=f"wqt{k}") for k in range(KT)]
        wkt = [cp.tile([128, LD], MF, name=f"wkt{k}", tag=f"wkt{k}") for k in range(KT)]
        wvt = [cp.tile([128, LD], MF, name=f"wvt{k}", tag=f"wvt{k}") for k in range(KT)]
        wo = [cp.tile([128, DM], MF, name=f"wo{k}", tag=f"wo{k}") for k in range(2)]
        bqt = [cp.tile([128, 1], F32, name=f"bqt{m}", tag=f"bqt{m}") for m in range(2)]
        ident = cp.tile([128, 128], F32, name="ident", tag="ident")
        ones1 = cp.tile([1, HD], F32, name="ones1", tag="ones1")
        hsrc = cp.tile([128, 512], MF, name="hsrc", tag="hsrc")
        warm = cp.tile([128, 1], F32, name="warm", tag="warm")
        for k in range(KT):
            nc.sync.dma_start(wqt[k][:], d["wq_t"][ts(k, 128), :])
            nc.sync.dma_start(wkt[k][:], d["wk_t"][ts(k, 128), :])
            nc.sync.dma_start(wvt[k][:], d["wv_t"][ts(k, 128), :])
        for k in range(2):
            nc.sync.dma_start(wo[k][:], d["wo_r"][ts(k, 128), :])
            nc.sync.dma_start(bqt[k][:], d["bq"][ts(k, 128), :])
        nc.sync.dma_start(ident[:], d["ident"][:, :])
        nc.sync.dma_start(ones1[:], d["ones1"][:, :])
        nc.vector.memset(hsrc[:], 0.001)
        # preload the Exp table set during the projection phase
        nc.vector.memset(warm[:], 0.0)
        nc.scalar.activation(warm[:], warm[:], EXP)

        # ---- persistent activations ----
        QTm = [pp.tile([128, S], MF, name=f"QT{m}", tag=f"QT{m}") for m in range(2)]
        KTm = [pp.tile([128, S], MF, name=f"KT{m}", tag=f"KT{m}") for m in range(2)]
        V = [pp.tile([128, LD], MF, name=f"V{j}", tag=f"V{j}") for j in range(QT)]
        CT = [pp.tile([128, S], MF, name=f"CT{m}", tag=f"CT{m}") for m in range(2)]
        IZ = [pp.tile([128, QT], F32, name=f"IZ{h}", tag=f"IZ{h}") for h in range(NHL)]

        xt = ctx.enter_context(tc.tile_pool(name="xt", bufs=10))
        izp = ctx.enter_context(tc.tile_pool(name="izp", bufs=2))
        stage = ctx.enter_context(tc.tile_pool(name="stage", bufs=4))
        expp = ctx.enter_context(tc.tile_pool(name="expp", bufs=3))
        smal = ctx.enter_context(tc.tile_pool(name="smal", bufs=2))
        zp = ctx.enter_context(tc.tile_pool(name="zp", bufs=3))

        def heater(pool, tag, nmm, label, bufs=None):
            # dense dummy matmul burst: forces the PE HAM monitor back to
            # the unthrottled clock after a pipeline bubble
            hp = pool.tile([128, 512], F32, name=f"heat{label}", tag=tag, bufs=bufs)
            for i in range(nmm):
                nc.tensor.matmul(
                    hp[:, 0:512], hsrc[:, 0:128], hsrc[:],
                    start=(i == 0), stop=(i == nmm - 1), skip_group_check=True,
                )

        # ================= Phase B: Q.T / K.T projections =================
        with tc.tile_pool(name="psb", bufs=2, space="PSUM") as psb:
            for name, dx, wt, out_t, bias in (
                ("k", "xk_t", wkt, KTm, None),
                ("q", "xq_t", wqt, QTm, bqt),
            ):
                xts = []
                for k in range(KT):
                    t = xt.tile([128, S], MF, name=f"x{name}{k}", tag="xt")
                    nc.sync.dma_start(t[:], d[dx][ts(k, 128), :])
                    xts.append(t)
                for n in range(NCH):
                    for m in range(2):
                        ps = psb.tile([128, 512], F32, name=f"ps{name}{n}{m}", tag="psq")
                        for k in range(KT):
                            nc.tensor.matmul(
                                ps[:],
                                wt[k][:, ts(m, 128)],
                                xts[k][:, ts(n, 512)],
                                start=(k == 0),
                                stop=(k == KT - 1),
                            )
                        if bias is not None:
                            nc.vector.tensor_scalar_add(
                                out_t[m][:, ts(n, 512)], ps[:], bias[m][:]
                            )
                        else:
                            nc.vector.tensor_copy(out_t[m][:, ts(n, 512)], ps[:])

        # ====== Phase C: alternating C1 head-pair / C3 head sections ======
        def v_units(pool):
            # B3: V = [2048, 256]; borrows pss-tag PSUM slots
            xvs = []
            for k in range(KT):
                t = xt.tile([128, S], MF, name=f"xv{k}", tag="xt")
                nc.sync.dma_start(t[:], d["xv_t"][ts(k, 128), :])
                xvs.append(t)
            for j in range(QT):
                ps = pool.tile([128, 1024], F32, name=f"psv{j}", tag="pss")
                for k in range(KT):
                    nc.tensor.matmul(
                        ps[:, 0:LD],
                        xvs[k][:, ts(j, 128)],
                        wvt[k][:],
                        start=(k == 0),
                        stop=(k == KT - 1),
                    )
                nc.vector.tensor_copy(V[j][:], ps[:, 0:LD])
                yield

        def c1_units(hp, pool):
            # heads ha (array rows 0-63) and hb (rows 64-127), packed
            ha, hb = 2 * hp, 2 * hp + 1
            qta = QTm[hp][0:64, :]
            kta = KTm[hp][0:64, :]
            qtb = QTm[hp][64:128, :]
            ktb = KTm[hp][64:128, :]
            for q in range(QT):
                sta = stage.tile([128, S], F32, name=f"st{ha}_{q}", tag="stage")
                stb = stage.tile([128, S], F32, name=f"st{hb}_{q}", tag="stage")
                zpa = zp.tile([128, 2], F32, name=f"zpa{ha}_{q}", tag="zpa")
                zpb = zp.tile([128, 2], F32, name=f"zpb{hb}_{q}", tag="zpb")
                for half in range(2):
                    psa = pool.tile([128, 1024], F32, name=f"psa{ha}{q}{half}", tag="pss")
                    psb = pool.tile([128, 1024], F32, name=f"psb{hb}{q}{half}", tag="pss")
                    for kc in range(2):
                        kk = half * 2 + kc
                        nc.tensor.matmul(
                            psa[:, ts(kc, 512)],
                            qta[:, ts(q, 128)],
                            kta[:, ts(kk, 512)],
                            start=True,
                            stop=True,
                        )
                        nc.tensor.matmul(
                            psb[:, ts(kc, 512)],
                            qtb[:, ts(q, 128)],
                            ktb[:, ts(kk, 512)],
                            start=True,
                            stop=True,
                        )
                    nc.scalar.activation(
                        sta[:, ts(half, 1024)], psa[:], EXP,
                        scale=0.125, accum_out=zpa[:, half : half + 1],
                    )
                    nc.scalar.activation(
                        stb[:, ts(half, 1024)], psb[:], EXP,
                        scale=0.125, accum_out=zpb[:, half : half + 1],
                    )
                for h, stx, zpx in ((ha, sta, zpa), (hb, stb, zpb)):
                    z1 = zp.tile([128, 1], F32, name=f"z1{h}_{q}", tag="z1")
                    nc.vector.reduce_sum(z1[:], zpx[:], axis=AXX)
                    nc.vector.reciprocal(IZ[h][:, q : q + 1], z1[:])
                    nc.vector.tensor_scalar_mul(stx[:], stx[:], IZ[h][:, q : q + 1])
                    nc.sync.dma_start(
                        d["attn_p"][h * S + q * 128 : h * S + (q + 1) * 128, :],
                        stx[:],
                    )
                yield

        def c3_head(h, pool):
            po = 64 * (h % 2)
            qth = QTm[h // 2][po : po + 64, :]
            kth = KTm[h // 2][po : po + 64, :]

            heater(pool, "psst", 16, f"c3{h}", bufs=3)

            # 1/Z -> [1, S] line (transpose + sbuf->sbuf reshape)
            pst = pool.tile([16, 128], F32, name=f"pst{h}", tag="pssm")
            nc.tensor.transpose(pst[:], IZ[h][:], ident[:])
            izrow = smal.tile([16, 128], F32, name=f"izrow{h}", tag="izrow")
            nc.vector.tensor_copy(izrow[:], pst[:])
            izline = izp.tile([1, S], F32, name=f"izline{h}", tag="izline")
            for rr in range(QT):
                nc.sync.dma_start(izline[0:1, ts(rr, 128)], izrow[rr : rr + 1, :])

            for n in range(NCH):
                pctx = pool.tile([64, 512], F32, name=f"pctx{h}{n}", tag="psctx")
                for jj in range(QT // 2):
                    pstt = pool.tile([128, 1024], F32, name=f"pstt{h}{n}{jj}", tag="psst", bufs=3)
                    for u in range(2):
                        j = 2 * jj + u
                        nc.tensor.matmul(
                            pstt[:, ts(u, 512)],
                            kth[:, ts(j, 128)],
                            qth[:, ts(n, 512)],
                            start=True,
                            stop=True,
                        )
                    et = expp.tile([128, 1024], MF, name=f"et{h}{n}{jj}", tag="expt")
                    nc.scalar.activation(et[:], pstt[:], EXP, scale=0.125)
                    for u in range(2):
                        j = 2 * jj + u
                        nc.tensor.matmul(
                            pctx[:],
                            V[j][:, ts(h, HD)],
                            et[:, ts(u, 512)],
                            start=(j == 0),
                            stop=(j == QT - 1),
                        )
                pbc = pool.tile([64, 512], F32, name=f"pbc{h}{n}", tag="pssm")
                nc.tensor.matmul(
                    pbc[:],
                    ones1[:],
                    izline[0:1, ts(n, 512)],
                    start=True,
                    stop=True,
                )
                bcs = smal.tile([64, 512], F32, name=f"bcs{h}{n}", tag="bcast")
                nc.vector.tensor_copy(bcs[:], pbc[:])
                nc.vector.tensor_mul(
                    CT[h // 2][po : po + 64, ts(n, 512)], pctx[:], bcs[:]
                )

        with tc.tile_pool(name="psc1a", bufs=4, space="PSUM") as pool:
            heater(pool, "pss", 24, "c1a")
            gv = v_units(pool)
            g1 = c1_units(0, pool)
            done_v = done_1 = False
            while not (done_v and done_1):
                if next(gv, StopIteration) is StopIteration:
                    done_v = True
                if next(g1, StopIteration) is StopIteration:
                    done_1 = True
            heater(pool, "pss", 16, "c1b")
            for _ in c1_units(1, pool):
                pass
        with tc.tile_pool(name="psc3a", bufs=1, space="PSUM") as pool:
            for h in range(NHL):
                c3_head(h, pool)

        # ================= Phase D: output projection =================
        with tc.tile_pool(name="psd", bufs=2, space="PSUM") as psd:
            for mt in range(QT):
                ost = smal.tile([128, DM], F32, name=f"ost{mt}", tag="outst")
                for nn in range(2):
                    pso = psd.tile([128, 512], F32, name=f"pso{mt}{nn}", tag="pso")
                    for kk in range(2):
                        nc.tensor.matmul(
                            pso[:],
                            CT[kk][:, ts(mt, 128)],
                            wo[kk][:, ts(nn, 512)],
                            start=(kk == 0),
                            stop=(kk == 1),
                        )
                    nc.vector.tensor_copy(ost[:, ts(nn, 512)], pso[:])
                nc.sync.dma_start(d["out_p"][ts(mt, 128), :], ost[:])


_NC = None


def _build():
    global _NC
    if _NC is not None:
        return _NC
    nc = bacc.Bacc("TRN2", target_bir_lowering=False, debug=False)
    d = {}
    for name, shape, dt_ in (
        ("xq_t", [DM, S], MF),
        ("xk_t", [DM, S], MF),
        ("xv_t", [DM, S], MF),
        ("wq_t", [DM, LD], MF),
        ("wk_t", [DM, LD], MF),
        ("wv_t", [DM, LD], MF),
        ("wo_r", [LD, DM], MF),
        ("bq", [LD, 1], F32),
        ("ident", [128, 128], F32),
        ("ones1", [1, HD], F32),
    ):
        d[name] = nc.dram_tensor(name, shape, dt_, kind="ExternalInput").ap()
    d["attn_p"] = nc.dram_tensor("attn_p", [NHL * S, S], F32, kind="ExternalOutput").ap()
    d["out_p"] = nc.dram_tensor("out_p", [S, DM], F32, kind="ExternalOutput").ap()

    with tile.TileContext(nc) as tc:
        _emit(nc, tc, d)
    nc.compile()
    _NC = nc
    return nc


def make_in_maps(query, key, value, wq_w, wq_b, wk_w, wk_b, wv_w, wv_b, wo_w, wo_b):
    f = np.float32
    ident = np.eye(128, dtype=f)
    ones1 = np.ones((1, HD), dtype=f)
    xt = {}
    for b in range(2):
        xt[("q", b)] = np.ascontiguousarray(np.asarray(query[b], dtype=f).T).astype(NP_MF)
        xt[("k", b)] = np.ascontiguousarray(np.asarray(key[b], dtype=f).T).astype(NP_MF)
        xt[("v", b)] = np.ascontiguousarray(np.asarray(value[b], dtype=f).T).astype(NP_MF)
    wqT = np.asarray(wq_w, dtype=f).T
    wkT = np.asarray(wk_w, dtype=f).T
    wvT = np.asarray(wv_w, dtype=f).T
    in_maps = []
    for c in range(NCORES):
        b, hg = divmod(c, 4)
        sl = slice(hg * LD, (hg + 1) * LD)
        in_maps.append(
            {
                "xq_t": xt[("q", b)],
                "xk_t": xt[("k", b)],
                "xv_t": xt[("v", b)],
                "wq_t": np.ascontiguousarray(wqT[:, sl]).astype(NP_MF),
                "wk_t": np.ascontiguousarray(wkT[:, sl]).astype(NP_MF),
                "wv_t": np.ascontiguousarray(wvT[:, sl]).astype(NP_MF),
                "wo_r": np.ascontiguousarray(np.asarray(wo_w, dtype=f)[:, sl].T).astype(NP_MF),
                "bq": np.ascontiguousarray(np.asarray(wq_b, dtype=f)[sl]).reshape(LD, 1),
                "ident": ident,
                "ones1": ones1,
            }
        )
    return in_maps


def kernel(query, key, value, wq_w, wq_b, wk_w, wk_b, wv_w, wv_b, wo_w, wo_b):
    _install_ntff_hook()
    nc = _build()
    in_maps = make_in_maps(
        query, key, value, wq_w, wq_b, wk_w, wk_b, wv_w, wv_b, wo_w, wo_b
    )
    res = bass_utils.run_bass_kernel_spmd(nc, in_maps, core_ids=list(range(NCORES)))
    kernel.last_exec_time_ns = res.exec_time_ns

    f = np.float32
    B, H = 2, 16
    attn = np.empty((B, H, S, S), dtype=f)
    out = np.zeros((B, S, DM), dtype=f)
    for c in range(NCORES):
        b, hg = divmod(c, 4)
        attn[b, hg * NHL : (hg + 1) * NHL] = res.results[c]["attn_p"].reshape(NHL, S, S)
        out[b] += res.results[c]["out_p"]
    # host-folded affine terms: value bias through wo, and wo bias
    out += (np.asarray(wv_b, dtype=f) @ np.asarray(wo_w, dtype=f).T + np.asarray(wo_b, dtype=f))[None, None, :]
    return out, attn
